# revision 1
# baseline (speedup 1.0000x reference)
"""Trainium2 Bass kernel for nn_Attention_13778255085887.

Dense multi-head attention block (EfficientViT-style):
  qkv 1x1 conv -> per-head softmax(q^T k * scale) -> v @ attn^T
  + depthwise conv(k=3) positional encoding on v -> proj 1x1 conv.

Shapes: B=8, dim=256, L=1024, heads=8, key_dim=16, head_dim=32.

Strategy: data-parallel over B across the 8 NeuronCores (zero collectives).
Per core, everything is computed in a "transposed-scores" layout so no
on-chip transposes are ever needed:
  - q/k are produced in a packed layout (4 heads per 128-partition tile,
    k in partitions 32g..32g+16, q in 32g+16..32g+32) so the tiny K=16
    score matmuls run 4-at-a-time via tile_position row groups.
  - S^T = k^T q is built per (head, j-chunk) directly with j on partitions,
    softmax-without-max (logits are provably tiny: |scale*S| < ~1.5), so
    exp is a single ScalarE ACTIVATE per (128, 2048) PSUM block.
  - v^T (AV stationary operand) is computed directly as x^T @ w_v^T by
    matmul, and v (natural, for the depthwise pe conv) by a second matmul.
  - AV out = (v^T)^T @ E accumulates over j in PSUM with 4 heads packed
    into one 128-partition tile via tile_position col groups; the softmax
    denominator d = sum_j E comes from parallel ones-vector matmuls.
  - 1/d is broadcast across the 32 head channels with a tiny ones x R
    matmul, y = av * R + pe(v) on VectorE, then the proj matmul.

ScalarE (exp over 8.4M elements/core) is the bottleneck engine; TensorE,
VectorE, GpSimd and DMA all hide underneath it.
"""

import os

import ml_dtypes
import numpy as np

import concourse.bass as bass
import concourse.mybir as mybir
import concourse.tile as tile
from concourse import bacc
from concourse.bass_utils import run_bass_kernel_spmd

BF16 = mybir.dt.bfloat16
F32 = mybir.dt.float32
AF = mybir.ActivationFunctionType
ALU = mybir.AluOpType

NH, KD, HD = 8, 16, 32
DIM, L, B = 256, 1024, 8
SCALE = KD ** -0.5  # 0.25


def _install_ntff_shim():
    """Optionally register the axon NTFF profiling hook (for trace=True).

    The container's antenv package lacks axon_hooks; recreate it and wire the
    ctypes-based hook from trn_agent_boot so neuron-profile exec times work.
    """
    import sys
    import types

    name = "antenv.axon_hooks"
    if name in sys.modules:
        return
    try:
        import antenv
        from trn_agent_boot.trn_boot import _ntff_profile_via_ctypes
    except ImportError:
        return
    hooks = types.ModuleType(name)
    hooks._the_hook = None
    hooks.set_axon_ntff_profile_hook = lambda h: setattr(hooks, "_the_hook", h)
    hooks.get_axon_ntff_profile_hook = lambda: hooks._the_hook
    sys.modules[name] = hooks
    antenv.axon_hooks = hooks
    so = "/opt/axon/libaxon_pjrt.so"
    if os.path.exists(so):
        hook = _ntff_profile_via_ctypes(so)
        if hook is not None:
            hooks.set_axon_ntff_profile_hook(hook)


def build_kernel() -> bass.Bass:
    nc = bacc.Bacc("TRN2", target_bir_lowering=False, debug=False, num_devices=8)

    # ---- DRAM I/O (per-core shard; weights replicated) ----
    x_d = nc.dram_tensor("x", (128, 2, 1024), BF16, kind="ExternalInput")
    wk_d = nc.dram_tensor("wk", (128, 2, 256), BF16, kind="ExternalInput")
    wq_d = nc.dram_tensor("wq", (128, 2, 256), BF16, kind="ExternalInput")
    wv_d = nc.dram_tensor("wv", (128, 2, 256), BF16, kind="ExternalInput")
    wpt_d = nc.dram_tensor("wpt", (128, 2, 256), BF16, kind="ExternalInput")
    bk_d = nc.dram_tensor("bk", (128, 2), F32, kind="ExternalInput")
    bq_d = nc.dram_tensor("bq", (128, 2), F32, kind="ExternalInput")
    bv_d = nc.dram_tensor("bv", (128, 2), F32, kind="ExternalInput")
    bvrow_d = nc.dram_tensor("bvrow", (1, 256), F32, kind="ExternalInput")
    wpe_d = nc.dram_tensor("wpe", (128, 2, 3), F32, kind="ExternalInput")
    bpe_d = nc.dram_tensor("bpe", (128, 2), F32, kind="ExternalInput")
    bproj_d = nc.dram_tensor("bproj", (128, 2), F32, kind="ExternalInput")
    onesb_d = nc.dram_tensor("onesb", (128, 1), BF16, kind="ExternalInput")
    onesf_d = nc.dram_tensor("onesf", (128, 32), F32, kind="ExternalInput")
    out_d = nc.dram_tensor("out", (128, 2, 1024), F32, kind="ExternalOutput")

    with tile.TileContext(nc) as tc:
        with (
            tc.tile_pool(name="const", bufs=1) as cpool,
            tc.tile_pool(name="work", bufs=3) as wpool,
            tc.tile_pool(name="epool", bufs=3) as epool,
            tc.tile_pool(name="ps_s", bufs=1, space="PSUM") as ps_s,
            tc.tile_pool(name="ps_av", bufs=1, space="PSUM") as ps_av,
            tc.tile_pool(name="ps_d", bufs=1, space="PSUM") as ps_d,
            tc.tile_pool(name="ps_misc", bufs=2, space="PSUM") as ps_misc,
        ):
            # ---- load constants / activations ----
            x_sb = cpool.tile([128, 2, 1024], BF16, tag="x")
            wk = cpool.tile([128, 2, 256], BF16, tag="wk")
            wq = cpool.tile([128, 2, 256], BF16, tag="wq")
            wv = cpool.tile([128, 2, 256], BF16, tag="wv")
            wpt = cpool.tile([128, 2, 256], BF16, tag="wpt")
            bk = cpool.tile([128, 2], F32, tag="bk")
            bq = cpool.tile([128, 2], F32, tag="bq")
            bv = cpool.tile([128, 2], F32, tag="bv")
            bvrow = cpool.tile([1, 256], F32, tag="bvrow")
            wpe = cpool.tile([128, 2, 3], F32, tag="wpe")
            bpe = cpool.tile([128, 2], F32, tag="bpe")
            bproj = cpool.tile([128, 2], F32, tag="bproj")
            onesb = cpool.tile([128, 1], BF16, tag="onesb")
            onesf = cpool.tile([128, 32], F32, tag="onesf")

            for sb, d in [
                (x_sb, x_d), (wk, wk_d), (wq, wq_d), (wv, wv_d), (wpt, wpt_d),
                (bk, bk_d), (bq, bq_d), (bv, bv_d), (bvrow, bvrow_d), (wpe, wpe_d),
                (bpe, bpe_d), (bproj, bproj_d), (onesb, onesb_d), (onesf, onesf_d),
            ]:
                nc.sync.dma_start(sb[:], d.ap())

            # persistent intermediates
            tk = cpool.tile([128, 2, 1024], BF16, tag="tk")        # packed k
            tq = cpool.tile([128, 2, 1024], BF16, tag="tq")        # packed q
            vnat = cpool.tile([128, 2, 1024], BF16, tag="vnat")    # v, natural
            vT = cpool.tile([128, 8, 256], BF16, tag="vT")         # v^T, j-chunked
            bvb = cpool.tile([128, 256], F32, tag="bvb")           # b_v row-broadcast
            y32 = cpool.tile([128, 2, 1024], F32, tag="y32")       # y = av*R + pe
            ybf = cpool.tile([128, 2, 1024], BF16, tag="ybf")
            zout = cpool.tile([128, 2, 1024], F32, tag="zout")

            # b_v broadcast across partitions for the v^T bias (free-dim bias)
            nc.gpsimd.partition_broadcast(bvb[:], bvrow[:], channels=128)

            # ---- phase 1: qkv projections ----
            for w_sb, b_sb, dst in [(wk, bk, tk), (wq, bq, tq)]:
                for t in range(2):
                    for n in range(2):
                        ps = ps_misc.tile([128, 512], F32, tag="mm")
                        for kc in range(2):
                            nc.tensor.matmul(
                                ps[:], w_sb[:, kc, t * 128:(t + 1) * 128],
                                x_sb[:, kc, n * 512:(n + 1) * 512],
                                start=(kc == 0), stop=(kc == 1),
                            )
                        nc.vector.tensor_scalar(
                            dst[:, t, n * 512:(n + 1) * 512], ps[:],
                            b_sb[:, t:t + 1], None, ALU.add,
                        )
            for t in range(2):
                for n in range(2):
                    ps = ps_misc.tile([128, 512], F32, tag="mm")
                    for kc in range(2):
                        nc.tensor.matmul(
                            ps[:], wv[:, kc, t * 128:(t + 1) * 128],
                            x_sb[:, kc, n * 512:(n + 1) * 512],
                            start=(kc == 0), stop=(kc == 1),
                        )
                    nc.vector.tensor_scalar(
                        vnat[:, t, n * 512:(n + 1) * 512], ps[:],
                        bv[:, t:t + 1], None, ALU.add,
                    )
            for jc in range(8):
                ps = ps_misc.tile([128, 512], F32, tag="mm")
                for kc in range(2):
                    nc.tensor.matmul(
                        ps[:, :256], x_sb[:, kc, jc * 128:(jc + 1) * 128],
                        wv[:, kc, :],
                        start=(kc == 0), stop=(kc == 1),
                    )
                nc.vector.tensor_tensor(
                    vT[:, jc, :], ps[:, :256], bvb[:], ALU.add,
                )

            # ---- phase 2: attention per pack group t (heads 4t..4t+3), i-half n ----
            for t in range(2):
                for n in range(2):
                    av_ps = ps_av.tile([128, 512], F32, tag="av")
                    d_ps = ps_d.tile([128, 512], F32, tag="d")
                    for jc in range(8):
                        s_ps = ps_s.tile([128, 2048], F32, tag="S")
                        for g in range(4):
                            nc.tensor.matmul(
                                s_ps[:, g * 512:(g + 1) * 512],
                                tk[32 * g:32 * g + 16, t, jc * 128:(jc + 1) * 128],
                                tq[32 * g:32 * g + 16, t, n * 512:(n + 1) * 512],
                                start=True, stop=True,
                                tile_position=(32 * g, 0),
                            )
                        e_sb = epool.tile([128, 2048], BF16, tag="E")
                        nc.scalar.activation(e_sb[:], s_ps[:], AF.Exp, scale=SCALE)
                        for g in range(4):
                            h = 4 * t + g
                            nc.tensor.matmul(
                                av_ps[32 * g:32 * g + 32, :],
                                vT[:, jc, 32 * h:32 * h + 32],
                                e_sb[:, g * 512:(g + 1) * 512],
                                start=(jc == 0), stop=(jc == 7),
                                tile_position=(0, 32 * g),
                                skip_group_check=True,
                            )
                            nc.tensor.matmul(
                                d_ps[32 * g:32 * g + 1, :],
                                onesb[:, :1],
                                e_sb[:, g * 512:(g + 1) * 512],
                                start=(jc == 0), stop=(jc == 7),
                                tile_position=(0, 32 * g),
                                skip_group_check=True,
                            )
                    # softmax denominator -> R = 1/d at partitions {0,32,64,96}
                    rall = wpool.tile([128, 512], F32, tag="rall")
                    nc.vector.reciprocal(rall[0:128:32, :], d_ps[0:128:32, :])
                    # broadcast R across 32 head channels: ones(1,32)^T @ R(1,512)
                    rb_ps = ps_misc.tile([128, 512], F32, tag="mm")
                    for g in range(4):
                        nc.tensor.matmul(
                            rb_ps[32 * g:32 * g + 32, :],
                            onesf[32 * g:32 * g + 1, :],
                            rall[32 * g:32 * g + 1, :],
                            start=True, stop=True,
                            tile_position=(32 * g, 32 * g),
                        )
                    rb_sb = wpool.tile([128, 512], F32, tag="rb")
                    nc.vector.tensor_copy(rb_sb[:], rb_ps[:])
                    nc.vector.tensor_tensor(
                        y32[:, t, n * 512:(n + 1) * 512], av_ps[:], rb_sb[:],
                        ALU.mult,
                    )

            # ---- phase 3: pe conv + proj ----
            for t in range(2):
                tmp_c = wpool.tile([128, 1024], F32, tag="pec")
                nc.vector.tensor_scalar(
                    tmp_c[:], vnat[:, t, :], wpe[:, t, 1:2], bpe[:, t:t + 1],
                    ALU.mult, ALU.add,
                )
                nc.vector.tensor_tensor(
                    y32[:, t, :], y32[:, t, :], tmp_c[:], ALU.add,
                )
                tmp_l = wpool.tile([128, 1024], F32, tag="pel")
                nc.vector.tensor_scalar(
                    tmp_l[:, :1023], vnat[:, t, :1023], wpe[:, t, 0:1], None,
                    ALU.mult,
                )
                nc.vector.tensor_tensor(
                    y32[:, t, 1:], y32[:, t, 1:], tmp_l[:, :1023], ALU.add,
                )
                tmp_r = wpool.tile([128, 1024], F32, tag="per")
                nc.vector.tensor_scalar(
                    tmp_r[:, :1023], vnat[:, t, 1:], wpe[:, t, 2:3], None,
                    ALU.mult,
                )
                nc.vector.tensor_tensor(
                    y32[:, t, :1023], y32[:, t, :1023], tmp_r[:, :1023], ALU.add,
                )
                nc.vector.tensor_copy(ybf[:, t, :], y32[:, t, :])

            for mo in range(2):
                for n in range(2):
                    ps = ps_misc.tile([128, 512], F32, tag="mm")
                    for kc in range(2):
                        nc.tensor.matmul(
                            ps[:], wpt[:, kc, mo * 128:(mo + 1) * 128],
                            ybf[:, kc, n * 512:(n + 1) * 512],
                            start=(kc == 0), stop=(kc == 1),
                        )
                    nc.vector.tensor_scalar(
                        zout[:, mo, n * 512:(n + 1) * 512], ps[:],
                        bproj[:, mo:mo + 1], None, ALU.add,
                    )
                    nc.sync.dma_start(
                        out_d.ap()[:, mo, n * 512:(n + 1) * 512],
                        zout[:, mo, n * 512:(n + 1) * 512],
                    )

    nc.compile()
    return nc


def pack_inputs(x, w_qkv, b_qkv, w_pe, b_pe, w_proj, b_proj):
    """Host-side packing of the full inputs into per-core in_maps."""
    bf16 = ml_dtypes.bfloat16
    f32 = np.float32

    # k/q packed layouts: tile t in {0,1}; partition m = 32*g + r; head h = 4t+g.
    # Only r < 16 is live (k channel r -> qkv row 64h+16+r; q channel r -> 64h+r);
    # r >= 16 columns are zero so both tiles stay 32-aligned per head.
    w_kA = np.zeros((256, 256), dtype=w_qkv.dtype)
    w_qA = np.zeros((256, 256), dtype=w_qkv.dtype)
    b_kP = np.zeros((128, 2), dtype=b_qkv.dtype)
    b_qP = np.zeros((128, 2), dtype=b_qkv.dtype)
    for t in range(2):
        for m in range(128):
            g, r = m // 32, m % 32
            h = 4 * t + g
            if r < 16:
                w_kA[:, t * 128 + m] = w_qkv[64 * h + 16 + r]
                w_qA[:, t * 128 + m] = w_qkv[64 * h + r]
                b_kP[m, t] = b_qkv[64 * h + 16 + r]
                b_qP[m, t] = b_qkv[64 * h + r]

    v_rows = np.array([64 * (c // 32) + 32 + c % 32 for c in range(256)])
    w_v = w_qkv[v_rows].T  # (256 d, 256 c)
    b_v = b_qkv[v_rows]

    def kpart(a):  # (256, F) -> (128, 2, F)
        return np.ascontiguousarray(a.reshape(2, 128, -1).transpose(1, 0, 2))

    def chan2(a):  # (256,) -> (128, 2)
        return np.ascontiguousarray(a.reshape(2, 128).T)

    common = {
        "wk": kpart(w_kA).astype(bf16),
        "wq": kpart(w_qA).astype(bf16),
        "wv": kpart(w_v).astype(bf16),
        "wpt": kpart(w_proj.T).astype(bf16),
        "bk": b_kP.astype(f32),
        "bq": b_qP.astype(f32),
        "bv": chan2(b_v).astype(f32),
        "bvrow": np.ascontiguousarray(b_v[None, :]).astype(f32),
        "wpe": kpart(w_pe[:, 0, :]).astype(f32),
        "bpe": chan2(b_pe).astype(f32),
        "bproj": chan2(b_proj).astype(f32),
        "onesb": np.ones((128, 1), dtype=bf16),
        "onesf": np.ones((128, 32), dtype=f32),
    }
    in_maps = []
    for b in range(B):
        m = dict(common)
        m["x"] = kpart(x[b]).astype(bf16)
        in_maps.append(m)
    return in_maps


_CACHE = {}


def kernel(x, w_qkv, b_qkv, w_pe, b_pe, w_proj, b_proj):
    x = np.asarray(x, dtype=np.float32)
    w_qkv = np.asarray(w_qkv, dtype=np.float32)
    b_qkv = np.asarray(b_qkv, dtype=np.float32)
    w_pe = np.asarray(w_pe, dtype=np.float32)
    b_pe = np.asarray(b_pe, dtype=np.float32)
    w_proj = np.asarray(w_proj, dtype=np.float32)
    b_proj = np.asarray(b_proj, dtype=np.float32)

    if "nc" not in _CACHE:
        _CACHE["nc"] = build_kernel()
    nc = _CACHE["nc"]

    in_maps = pack_inputs(x, w_qkv, b_qkv, w_pe, b_pe, w_proj, b_proj)

    trace = os.environ.get("BASS_KERNEL_TRACE", "") == "1"
    if trace:
        _install_ntff_shim()
    res = run_bass_kernel_spmd(
        nc, in_maps, core_ids=list(range(B)), trace=trace,
    )
    if trace:
        _CACHE["last_result"] = res

    out = np.empty((B, DIM, L), dtype=np.float32)
    for b in range(B):
        z = res.results[b]["out"]  # (128, 2, 1024)
        out[b] = z.transpose(1, 0, 2).reshape(DIM, L)
    return out


# revision 2
# speedup vs baseline: 1.2278x; 1.2278x over previous
"""Trainium2 Bass kernel for nn_Attention_13778255085887.

Dense multi-head attention block (EfficientViT-style):
  qkv 1x1 conv -> per-head softmax(q^T k * scale) -> v @ attn^T
  + depthwise conv(k=3) positional encoding on v -> proj 1x1 conv.

Shapes: B=8, dim=256, L=1024, heads=8, key_dim=16, head_dim=32.

Strategy: data-parallel over B across the 8 NeuronCores (zero collectives).
Per core, everything is computed in a "transposed-scores" layout so no
on-chip transposes are ever needed:
  - q/k are produced in a packed layout (4 heads per 128-partition tile,
    k in partitions 32g..32g+16, q in 32g+16..32g+32) so the tiny K=16
    score matmuls run 4-at-a-time via tile_position row groups.
  - S^T = k^T q is built per (head, j-chunk) directly with j on partitions,
    softmax-without-max (logits are provably tiny: |scale*S| < ~1.5), so
    exp is a single ScalarE ACTIVATE per (128, 2048) PSUM block.
  - v^T (AV stationary operand) is computed directly as x^T @ w_v^T by
    matmul, and v (natural, for the depthwise pe conv) by a second matmul.
  - AV out = (v^T)^T @ E accumulates over j in PSUM with 4 heads packed
    into one 128-partition tile via tile_position col groups; the softmax
    denominator d = sum_j E comes from parallel ones-vector matmuls.
  - 1/d is broadcast across the 32 head channels with a tiny ones x R
    matmul, y = av * R + pe(v) on VectorE, then the proj matmul.

ScalarE (exp over 8.4M elements/core) is the bottleneck engine; TensorE,
VectorE, GpSimd and DMA all hide underneath it.
"""

import os

import ml_dtypes
import numpy as np

import concourse.bass as bass
import concourse.mybir as mybir
import concourse.tile as tile
from concourse import bacc
from concourse.bass_utils import run_bass_kernel_spmd

BF16 = mybir.dt.bfloat16
F32 = mybir.dt.float32
AF = mybir.ActivationFunctionType
ALU = mybir.AluOpType

NH, KD, HD = 8, 16, 32
DIM, L, B = 256, 1024, 8
SCALE = KD ** -0.5  # 0.25


def _install_ntff_shim():
    """Optionally register the axon NTFF profiling hook (for trace=True).

    The container's antenv package lacks axon_hooks; recreate it and wire the
    ctypes-based hook from trn_agent_boot so neuron-profile exec times work.
    """
    import sys
    import types

    name = "antenv.axon_hooks"
    if name in sys.modules:
        return
    try:
        import antenv
        from trn_agent_boot.trn_boot import _ntff_profile_via_ctypes
    except ImportError:
        return
    hooks = types.ModuleType(name)
    hooks._the_hook = None
    hooks.set_axon_ntff_profile_hook = lambda h: setattr(hooks, "_the_hook", h)
    hooks.get_axon_ntff_profile_hook = lambda: hooks._the_hook
    sys.modules[name] = hooks
    antenv.axon_hooks = hooks
    so = "/opt/axon/libaxon_pjrt.so"
    if os.path.exists(so):
        hook = _ntff_profile_via_ctypes(so)
        if hook is not None:
            hooks.set_axon_ntff_profile_hook(hook)


def build_kernel() -> bass.Bass:
    nc = bacc.Bacc("TRN2", target_bir_lowering=False, debug=False, num_devices=8)

    # ---- DRAM I/O (per-core shard; weights replicated) ----
    x_d = nc.dram_tensor("x", (128, 2, 1024), BF16, kind="ExternalInput")
    wk_d = nc.dram_tensor("wk", (128, 2, 256), BF16, kind="ExternalInput")
    wq_d = nc.dram_tensor("wq", (128, 2, 256), BF16, kind="ExternalInput")
    wv_d = nc.dram_tensor("wv", (128, 2, 256), BF16, kind="ExternalInput")
    wpt_d = nc.dram_tensor("wpt", (128, 2, 256), BF16, kind="ExternalInput")
    bk_d = nc.dram_tensor("bk", (128, 2), F32, kind="ExternalInput")
    bq_d = nc.dram_tensor("bq", (128, 2), F32, kind="ExternalInput")
    bv_d = nc.dram_tensor("bv", (128, 2), F32, kind="ExternalInput")
    bvrow_d = nc.dram_tensor("bvrow", (1, 256), F32, kind="ExternalInput")
    wpe_d = nc.dram_tensor("wpe", (128, 2, 3), F32, kind="ExternalInput")
    bpe_d = nc.dram_tensor("bpe", (128, 2), F32, kind="ExternalInput")
    bproj_d = nc.dram_tensor("bproj", (128, 2), F32, kind="ExternalInput")
    onesb_d = nc.dram_tensor("onesb", (128, 1), BF16, kind="ExternalInput")
    onesf_d = nc.dram_tensor("onesf", (128, 32), F32, kind="ExternalInput")
    out_d = nc.dram_tensor("out", (128, 2, 1024), F32, kind="ExternalOutput")

    with tile.TileContext(nc) as tc:
        with (
            tc.tile_pool(name="const", bufs=1) as cpool,
            tc.tile_pool(name="work", bufs=3) as wpool,
            tc.tile_pool(name="epool", bufs=3) as epool,
            tc.tile_pool(name="ps_s", bufs=1, space="PSUM") as ps_s,
            tc.tile_pool(name="ps_av", bufs=1, space="PSUM") as ps_av,
            tc.tile_pool(name="ps_d", bufs=1, space="PSUM") as ps_d,
            tc.tile_pool(name="ps_misc", bufs=2, space="PSUM") as ps_misc,
        ):
            # ---- load constants / activations ----
            x_sb = cpool.tile([128, 2, 1024], BF16, tag="x")
            wk = cpool.tile([128, 2, 256], BF16, tag="wk")
            wq = cpool.tile([128, 2, 256], BF16, tag="wq")
            wv = cpool.tile([128, 2, 256], BF16, tag="wv")
            wpt = cpool.tile([128, 2, 256], BF16, tag="wpt")
            bk = cpool.tile([128, 2], F32, tag="bk")
            bq = cpool.tile([128, 2], F32, tag="bq")
            bv = cpool.tile([128, 2], F32, tag="bv")
            bvrow = cpool.tile([1, 256], F32, tag="bvrow")
            wpe = cpool.tile([128, 2, 3], F32, tag="wpe")
            bpe = cpool.tile([128, 2], F32, tag="bpe")
            bproj = cpool.tile([128, 2], F32, tag="bproj")
            onesb = cpool.tile([128, 1], BF16, tag="onesb")
            onesf = cpool.tile([128, 32], F32, tag="onesf")

            for sb, d in [
                (x_sb, x_d), (wk, wk_d), (wq, wq_d), (wv, wv_d), (wpt, wpt_d),
                (bk, bk_d), (bq, bq_d), (bv, bv_d), (bvrow, bvrow_d), (wpe, wpe_d),
                (bpe, bpe_d), (bproj, bproj_d), (onesb, onesb_d), (onesf, onesf_d),
            ]:
                nc.sync.dma_start(sb[:], d.ap())

            # persistent intermediates
            tk = cpool.tile([128, 2, 1024], BF16, tag="tk")        # packed k
            tq = cpool.tile([128, 2, 1024], BF16, tag="tq")        # packed q
            vnat = cpool.tile([128, 2, 1024], BF16, tag="vnat")    # v, natural
            vT = cpool.tile([128, 8, 256], BF16, tag="vT")         # v^T, j-chunked
            bvb = cpool.tile([128, 256], F32, tag="bvb")           # b_v row-broadcast
            y32 = cpool.tile([128, 2, 1024], F32, tag="y32")       # y = av*R + pe
            ybf = cpool.tile([128, 2, 1024], BF16, tag="ybf")
            zout = cpool.tile([128, 2, 1024], F32, tag="zout")

            # b_v broadcast across partitions for the v^T bias (free-dim bias)
            nc.gpsimd.partition_broadcast(bvb[:], bvrow[:], channels=128)

            # ---- phase 1: qkv projections ----
            for w_sb, b_sb, dst in [(wk, bk, tk), (wq, bq, tq)]:
                for t in range(2):
                    for n in range(2):
                        ps = misc_ps()
                        for kc in range(2):
                            nc.tensor.matmul(
                                ps[:], w_sb[:, kc, t * 128:(t + 1) * 128],
                                x_sb[:, kc, n * 512:(n + 1) * 512],
                                start=(kc == 0), stop=(kc == 1),
                            )
                        nc.vector.tensor_scalar(
                            dst[:, t, n * 512:(n + 1) * 512], ps[:],
                            b_sb[:, t:t + 1], None, ALU.add,
                        )
            for t in range(2):
                for n in range(2):
                    ps = misc_ps()
                    for kc in range(2):
                        nc.tensor.matmul(
                            ps[:], wv[:, kc, t * 128:(t + 1) * 128],
                            x_sb[:, kc, n * 512:(n + 1) * 512],
                            start=(kc == 0), stop=(kc == 1),
                        )
                    nc.vector.tensor_scalar(
                        vnat[:, t, n * 512:(n + 1) * 512], ps[:],
                        bv[:, t:t + 1], None, ALU.add,
                    )
            for jc in range(8):
                ps = misc_ps()
                for kc in range(2):
                    nc.tensor.matmul(
                        ps[:, :256], x_sb[:, kc, jc * 128:(jc + 1) * 128],
                        wv[:, kc, :],
                        start=(kc == 0), stop=(kc == 1),
                    )
                nc.vector.tensor_tensor(
                    vT[:, jc, :], ps[:, :256], bvb[:], ALU.add,
                )

            # ---- phase 2: attention per pack group t (heads 4t..4t+3), i-half n ----
            for t in range(2):
                for n in range(2):
                    av_ps = ps_av.tile([128, 512], F32, tag="av")
                    d_ps = ps_d.tile([128, 512], F32, tag="d")
                    for jc in range(8):
                        s_ps = ps_s.tile([128, 2048], F32, tag="S")
                        for g in range(4):
                            nc.tensor.matmul(
                                s_ps[:, g * 512:(g + 1) * 512],
                                tk[32 * g:32 * g + 16, t, jc * 128:(jc + 1) * 128],
                                tq[32 * g:32 * g + 16, t, n * 512:(n + 1) * 512],
                                start=True, stop=True,
                                tile_position=(32 * g, 0),
                            )
                        e_sb = epool.tile([128, 2048], BF16, tag="E")
                        nc.scalar.activation(e_sb[:], s_ps[:], AF.Exp, scale=SCALE)
                        for g in range(4):
                            h = 4 * t + g
                            nc.tensor.matmul(
                                av_ps[32 * g:32 * g + 32, :],
                                vT[:, jc, 32 * h:32 * h + 32],
                                e_sb[:, g * 512:(g + 1) * 512],
                                start=(jc == 0), stop=(jc == 7),
                                tile_position=(0, 32 * g),
                                skip_group_check=True,
                            )
                            nc.tensor.matmul(
                                d_ps[32 * g:32 * g + 1, :],
                                onesb[:, :1],
                                e_sb[:, g * 512:(g + 1) * 512],
                                start=(jc == 0), stop=(jc == 7),
                                tile_position=(0, 32 * g),
                                skip_group_check=True,
                            )
                    # softmax denominator -> R = 1/d at partitions {0,32,64,96}
                    rall = wpool.tile([128, 512], F32, tag="rall")
                    nc.vector.reciprocal(rall[0:128:32, :], d_ps[0:128:32, :])
                    # broadcast R across 32 head channels: ones(1,32)^T @ R(1,512)
                    rb_ps = misc_ps()
                    for g in range(4):
                        nc.tensor.matmul(
                            rb_ps[32 * g:32 * g + 32, :],
                            onesf[32 * g:32 * g + 1, :],
                            rall[32 * g:32 * g + 1, :],
                            start=True, stop=True,
                            tile_position=(32 * g, 32 * g),
                        )
                    rb_sb = wpool.tile([128, 512], F32, tag="rb")
                    nc.vector.tensor_copy(rb_sb[:], rb_ps[:])
                    nc.vector.tensor_tensor(
                        y32[:, t, n * 512:(n + 1) * 512], av_ps[:], rb_sb[:],
                        ALU.mult,
                    )

            # ---- phase 3: pe conv + proj ----
            for t in range(2):
                tmp_c = wpool.tile([128, 1024], F32, tag="pec")
                nc.vector.tensor_scalar(
                    tmp_c[:], vnat[:, t, :], wpe[:, t, 1:2], bpe[:, t:t + 1],
                    ALU.mult, ALU.add,
                )
                nc.vector.tensor_tensor(
                    y32[:, t, :], y32[:, t, :], tmp_c[:], ALU.add,
                )
                tmp_l = wpool.tile([128, 1024], F32, tag="pel")
                nc.vector.tensor_scalar(
                    tmp_l[:, :1023], vnat[:, t, :1023], wpe[:, t, 0:1], None,
                    ALU.mult,
                )
                nc.vector.tensor_tensor(
                    y32[:, t, 1:], y32[:, t, 1:], tmp_l[:, :1023], ALU.add,
                )
                tmp_r = wpool.tile([128, 1024], F32, tag="per")
                nc.vector.tensor_scalar(
                    tmp_r[:, :1023], vnat[:, t, 1:], wpe[:, t, 2:3], None,
                    ALU.mult,
                )
                nc.vector.tensor_tensor(
                    y32[:, t, :1023], y32[:, t, :1023], tmp_r[:, :1023], ALU.add,
                )
                nc.vector.tensor_copy(ybf[:, t, :], y32[:, t, :])

            for mo in range(2):
                for n in range(2):
                    ps = misc_ps()
                    for kc in range(2):
                        nc.tensor.matmul(
                            ps[:], wpt[:, kc, mo * 128:(mo + 1) * 128],
                            ybf[:, kc, n * 512:(n + 1) * 512],
                            start=(kc == 0), stop=(kc == 1),
                        )
                    nc.vector.tensor_scalar(
                        zout[:, mo, n * 512:(n + 1) * 512], ps[:],
                        bproj[:, mo:mo + 1], None, ALU.add,
                    )
                    nc.sync.dma_start(
                        out_d.ap()[:, mo, n * 512:(n + 1) * 512],
                        zout[:, mo, n * 512:(n + 1) * 512],
                    )

    nc.compile()
    return nc


def pack_inputs(x, w_qkv, b_qkv, w_pe, b_pe, w_proj, b_proj):
    """Host-side packing of the full inputs into per-core in_maps."""
    bf16 = ml_dtypes.bfloat16
    f32 = np.float32

    # k/q packed layouts: tile t in {0,1}; partition m = 32*g + r; head h = 4t+g.
    # Only r < 16 is live (k channel r -> qkv row 64h+16+r; q channel r -> 64h+r);
    # r >= 16 columns are zero so both tiles stay 32-aligned per head.
    w_kA = np.zeros((256, 256), dtype=w_qkv.dtype)
    w_qA = np.zeros((256, 256), dtype=w_qkv.dtype)
    b_kP = np.zeros((128, 2), dtype=b_qkv.dtype)
    b_qP = np.zeros((128, 2), dtype=b_qkv.dtype)
    for t in range(2):
        for m in range(128):
            g, r = m // 32, m % 32
            h = 4 * t + g
            if r < 16:
                w_kA[:, t * 128 + m] = w_qkv[64 * h + 16 + r]
                w_qA[:, t * 128 + m] = w_qkv[64 * h + r]
                b_kP[m, t] = b_qkv[64 * h + 16 + r]
                b_qP[m, t] = b_qkv[64 * h + r]

    v_rows = np.array([64 * (c // 32) + 32 + c % 32 for c in range(256)])
    w_v = w_qkv[v_rows].T  # (256 d, 256 c)
    b_v = b_qkv[v_rows]

    def kpart(a):  # (256, F) -> (128, 2, F)
        return np.ascontiguousarray(a.reshape(2, 128, -1).transpose(1, 0, 2))

    def chan2(a):  # (256,) -> (128, 2)
        return np.ascontiguousarray(a.reshape(2, 128).T)

    common = {
        "wk": kpart(w_kA).astype(bf16),
        "wq": kpart(w_qA).astype(bf16),
        "wv": kpart(w_v).astype(bf16),
        "wpt": kpart(w_proj.T).astype(bf16),
        "bk": b_kP.astype(f32),
        "bq": b_qP.astype(f32),
        "bv": chan2(b_v).astype(f32),
        "bvrow": np.ascontiguousarray(b_v[None, :]).astype(f32),
        "wpe": kpart(w_pe[:, 0, :]).astype(f32),
        "bpe": chan2(b_pe).astype(f32),
        "bproj": chan2(b_proj).astype(f32),
        "onesb": np.ones((128, 1), dtype=bf16),
        "onesf": np.ones((128, 32), dtype=f32),
    }
    in_maps = []
    for b in range(B):
        m = dict(common)
        m["x"] = kpart(x[b]).astype(bf16)
        in_maps.append(m)
    return in_maps


_CACHE = {}


def kernel(x, w_qkv, b_qkv, w_pe, b_pe, w_proj, b_proj):
    x = np.asarray(x, dtype=np.float32)
    w_qkv = np.asarray(w_qkv, dtype=np.float32)
    b_qkv = np.asarray(b_qkv, dtype=np.float32)
    w_pe = np.asarray(w_pe, dtype=np.float32)
    b_pe = np.asarray(b_pe, dtype=np.float32)
    w_proj = np.asarray(w_proj, dtype=np.float32)
    b_proj = np.asarray(b_proj, dtype=np.float32)

    if "nc" not in _CACHE:
        _CACHE["nc"] = build_kernel()
    nc = _CACHE["nc"]

    in_maps = pack_inputs(x, w_qkv, b_qkv, w_pe, b_pe, w_proj, b_proj)

    trace = os.environ.get("BASS_KERNEL_TRACE", "") == "1"
    if trace:
        _install_ntff_shim()
    res = run_bass_kernel_spmd(
        nc, in_maps, core_ids=list(range(B)), trace=trace,
    )
    if trace:
        _CACHE["last_result"] = res

    out = np.empty((B, DIM, L), dtype=np.float32)
    for b in range(B):
        z = res.results[b]["out"]  # (128, 2, 1024)
        out[b] = z.transpose(1, 0, 2).reshape(DIM, L)
    return out
